# revision 11
# baseline (speedup 1.0000x reference)
"""BiMambaBlock on 8 Trainium2 NeuronCores.

Sharding: core c = (batch b, direction d, d_inner-half h) with
b = c>>2, d = (c>>1)&1, h = c&1.  Every core runs the same program on
different data (weights sliced/permuted per core on the host):

  - host feeds x[b].T in bf16 (flipped along L for bwd cores, padded
    with 3 leading zero cols for the causal conv), so the device always
    runs a *forward* mamba mixer in channels-on-partitions layout [d, L].
  - each core computes the full xc = silu(conv(x @ in_w_xi)) over all
    1024 channels (so the xproj contraction over d_inner stays local,
    no collectives), but scans only its 512-channel half (the host
    permutes weights so the own half is always channel blocks 0-3).
  - out_proj and the final fuse matmul are folded on the host into one
    [512ch, 512dm] weight; each core emits a partial [512dm, L] f32
    which the host transposes/flips/sums.

v2 structure (DVE is the bottleneck: 64 tensor_tensor_scans at
~2.16 ns/elem = 283us/core are irreducible; everything else must
overlap under them or leave DVE):
  - phase 2 loops blk-outer / n-inner; y = sum_n h_n*C_n accumulates in
    PSUM via identity-lhsT matmuls on the (idle) PE instead of 60 DVE
    tensor_tensor adds.
  - B_n/C_n rows are broadcast across partitions once (selector
    matmuls + ACT drains), pipelined two n ahead inside blk0's n-loop,
    and the [128,L] bf16 broadcast tiles stay resident for blks 1-3.
  - conv taps 0/2 run as one tensor_scalar + one fused
    scalar_tensor_tensor on DVE; taps 1/3 on ACT (odd byte offsets
    would drop DVE to 1x mode).
  - z-gate matmuls issue after the dt chain, so they fill PE/ACT time
    under the first scans instead of delaying them.
  - scan writes in-place over its dA input and p reuses the dBx buffer
    (SBUF is within ~2 KiB/partition of full with 32 broadcast tiles).
"""
import os
import sys

for _p in ("/opt/trn_rl_repo",):
    if os.path.isdir(_p) and _p not in sys.path:
        sys.path.insert(0, _p)

from contextlib import ExitStack

import ml_dtypes
import numpy as np

from concourse import bass, mybir, tile
from concourse.bass_utils import run_bass_kernel_spmd

F32 = mybir.dt.float32
BF16 = mybir.dt.bfloat16
AF = mybir.ActivationFunctionType
OP = mybir.AluOpType

D_MODEL = 512
D_INNER = 1024
DH = 512
N_STATE = 16
D_CONV = 4
DT_RANK = 32
B = 2
L = 2048
LP = L + 3

NBLK_F = D_INNER // 128  # 8 channel blocks for conv/xproj
NBLK_H = DH // 128       # 4 scan blocks

NCH = L // 512           # 512-wide matmul N-chunks

BF16NP = ml_dtypes.bfloat16


def _build_program():
    nc = bass.Bass(trn_type="TRN2", target_bir_lowering=False, debug=False)

    xT_d = nc.dram_tensor("xT", [128, 4 * LP], BF16, kind="ExternalInput")
    w_in_d = nc.dram_tensor("w_in", [128, 4 * 1536], BF16, kind="ExternalInput")
    conv_w_d = nc.dram_tensor("conv_w", [128, NBLK_F * D_CONV], F32, kind="ExternalInput")
    conv_b_d = nc.dram_tensor("conv_b", [128, NBLK_F], F32, kind="ExternalInput")
    xproj_w_d = nc.dram_tensor("xproj_w", [128, NBLK_F * 64], BF16, kind="ExternalInput")
    dt_w_d = nc.dram_tensor("dt_w", [DT_RANK, DH], BF16, kind="ExternalInput")
    dt_b_d = nc.dram_tensor("dt_b", [128, NBLK_H], F32, kind="ExternalInput")
    A_d = nc.dram_tensor("A", [128, NBLK_H * N_STATE], F32, kind="ExternalInput")
    D_d = nc.dram_tensor("D", [128, NBLK_H], F32, kind="ExternalInput")
    w_out_d = nc.dram_tensor("w_out", [128, 4 * D_MODEL], BF16, kind="ExternalInput")
    bcsel_d = nc.dram_tensor("bcsel", [N_STATE, N_STATE * 128], BF16,
                             kind="ExternalInput")
    ident_d = nc.dram_tensor("ident", [128, 128], BF16, kind="ExternalInput")
    out_d = nc.dram_tensor("out_part", [D_MODEL, L], F32, kind="ExternalOutput")

    with tile.TileContext(nc) as tc, ExitStack() as ctx:
        # ---------------- persistent small weights ----------------
        wp = ctx.enter_context(tc.tile_pool(name="weights", bufs=1))

        conv_w = wp.tile([128, NBLK_F * D_CONV], F32, tag="conv_w")
        conv_b = wp.tile([128, NBLK_F], F32, tag="conv_b")
        xproj_w = wp.tile([128, NBLK_F * 64], BF16, tag="xproj_w")
        dt_w = wp.tile([DT_RANK, DH], BF16, tag="dt_w")
        dt_b = wp.tile([128, NBLK_H], F32, tag="dt_b")
        A_sb = wp.tile([128, NBLK_H * N_STATE], F32, tag="A")
        D_sb = wp.tile([128, NBLK_H], F32, tag="D")
        w_out = wp.tile([128, 4 * D_MODEL], BF16, tag="w_out")
        bcsel = wp.tile([N_STATE, N_STATE * 128], BF16, tag="bcsel")
        ident = wp.tile([128, 128], BF16, tag="ident")

        # long-lived activation tiles
        glob = ctx.enter_context(tc.tile_pool(name="glob", bufs=1))
        xc_t = [glob.tile([128, L], BF16, tag=f"xc{i}", name=f"xc{i}")
                for i in range(NBLK_H)]  # own-half xc; reused as gate output
        gz_t = [glob.tile([128, L], BF16, tag=f"gz{i}", name=f"gz{i}")
                for i in range(NBLK_H)]
        B_sb = glob.tile([N_STATE, L], BF16, tag="Brows")
        C_sb = glob.tile([N_STATE, L], BF16, tag="Crows")
        dbc = glob.tile([64, L], BF16, tag="dbc")

        # PSUM pools: pio lives the whole kernel (2 banks); phase-1-only
        # banks (pio1 + pdbc, 6) close before the phase-2 y accumulator
        # (4 banks) opens.
        pio = ctx.enter_context(tc.tile_pool(name="pio", bufs=2, space="PSUM"))

        with tc.tile_pool(name="pin", bufs=1) as pin, \
             tc.tile_pool(name="ph1b", bufs=2) as ph1b, \
             tc.tile_pool(name="ph1c", bufs=2) as ph1c, \
             tc.tile_pool(name="pio1", bufs=2, space="PSUM") as pio1, \
             tc.tile_pool(name="pdbc", bufs=1, space="PSUM") as pdbc:
            xT = pin.tile([128, 4 * LP], BF16, tag="xT")
            w_in = pin.tile([128, 4 * 1536], BF16, tag="w_in")

            # fine-grained DMA order: first matmul's operands land first
            for kb in range(4):
                nc.sync.dma_start(xT[:, kb * LP:kb * LP + 515],
                                  xT_d[:, kb * LP:kb * LP + 515])
            for kb in range(4):
                nc.sync.dma_start(w_in[:, kb * 1536:kb * 1536 + 768],
                                  w_in_d[:, kb * 1536:kb * 1536 + 768])
            for nch in range(1, 4):
                for kb in range(4):
                    o = kb * LP + 3 + nch * 512
                    nc.sync.dma_start(xT[:, o:o + 512], xT_d[:, o:o + 512])
            for kb in range(4):
                nc.sync.dma_start(w_in[:, kb * 1536 + 768:(kb + 1) * 1536],
                                  w_in_d[:, kb * 1536 + 768:(kb + 1) * 1536])
            for t, d in [(conv_w, conv_w_d),
                         (conv_b, conv_b_d), (xproj_w, xproj_w_d),
                         (dt_w, dt_w_d), (dt_b, dt_b_d), (A_sb, A_d),
                         (D_sb, D_d), (w_out, w_out_d), (bcsel, bcsel_d),
                         (ident, ident_d)]:
                nc.sync.dma_start(t[:], d[:])

            xT_v = xT[:].rearrange("p (k l) -> p k l", k=4)
            w_in_v = w_in[:].rearrange("p (k m) -> p k m", k=4)
            xproj_v = xproj_w[:].rearrange("p (k f) -> p k f", k=NBLK_F)

            # xproj accumulators persist across the whole blk loop
            ps_dbc = [pdbc.tile([64, 512], F32, tag=f"pdbc{nch}",
                                name=f"ps_dbc{nch}") for nch in range(NCH)]

            # ---------------- phase 1: xc / xproj / dt ----------------
            for blk in range(NBLK_F):
                xi = ph1b.tile([128, LP], BF16, tag="xi", name="xi")
                m0 = blk * 128
                for gi, nch in enumerate(range(0, LP, 512)):
                    w = min(512, LP - nch)
                    ps = pio1.tile([128, 512], F32, tag="pio1", name="ps_in")
                    for kb in range(4):
                        nc.tensor.matmul(
                            ps[:, 0:w],
                            lhsT=w_in_v[:, kb, m0:m0 + 128],
                            rhs=xT_v[:, kb, nch:nch + w],
                            start=(kb == 0), stop=(kb == 3),
                        )
                    nc.vector.tensor_copy(xi[:, nch:nch + w], ps[:, 0:w])
                # conv: taps 0/2 on DVE (aligned, TS then fused STT),
                # taps 1/3 on ACT, two DVE adds to combine
                acc = ph1c.tile([128, L], BF16, tag="ct0", name="ct0")
                nc.vector.tensor_scalar_mul(
                    acc[:], xi[:, 0:L], conv_w[:, blk * 4:blk * 4 + 1])
                nc.vector.scalar_tensor_tensor(
                    acc[:], xi[:, 2:2 + L], conv_w[:, blk * 4 + 2:blk * 4 + 3],
                    acc[:], OP.mult, OP.add)
                t1 = ph1c.tile([128, L], BF16, tag="ct1", name="ct1")
                nc.scalar.mul(t1[:], xi[:, 1:1 + L],
                              conv_w[:, blk * 4 + 1:blk * 4 + 2])
                t3 = ph1c.tile([128, L], BF16, tag="ct3", name="ct3")
                nc.scalar.mul(t3[:], xi[:, 3:3 + L],
                              conv_w[:, blk * 4 + 3:blk * 4 + 4])
                nc.vector.tensor_tensor(t1[:], t1[:], t3[:], OP.add)
                nc.vector.tensor_tensor(acc[:], acc[:], t1[:], OP.add)
                if blk < NBLK_H:
                    xc = xc_t[blk]
                else:
                    xc = ph1b.tile([128, L], BF16, tag="xcO", name=f"xcO{blk}")
                nc.scalar.activation(xc[:], acc[:], AF.Silu,
                                     bias=conv_b[:, blk:blk + 1])
                if blk >= NBLK_H:
                    xc_t.append(xc)

                # xproj contribution of this block (accumulates over blks)
                for nch in range(NCH):
                    nc.tensor.matmul(
                        ps_dbc[nch][:], lhsT=xproj_v[:, blk, :],
                        rhs=xc[:, nch * 512:(nch + 1) * 512],
                        start=(blk == 0), stop=(blk == NBLK_F - 1),
                    )
                    if blk == NBLK_F - 1:
                        nc.scalar.copy(dbc[:, nch * 512:(nch + 1) * 512],
                                       ps_dbc[nch][:])

            nc.sync.dma_start(B_sb[:], dbc[32:48, :])
            nc.sync.dma_start(C_sb[:], dbc[48:64, :])

            # z gate: off the critical path (needed only at blk0's gate,
            # ~100us after the first scan starts)
            for blk in range(NBLK_H):
                for nch in range(NCH):
                    ps = pio1.tile([128, 512], F32, tag="pio1", name="ps_z")
                    for kb in range(4):
                        nc.tensor.matmul(
                            ps[:],
                            lhsT=w_in_v[:, kb, 1024 + blk * 128:1024 + (blk + 1) * 128],
                            rhs=xT_v[:, kb, 3 + nch * 512:3 + (nch + 1) * 512],
                            start=(kb == 0), stop=(kb == 3),
                        )
                    nc.scalar.activation(gz_t[blk][:, nch * 512:(nch + 1) * 512],
                                         ps[:], AF.Silu)

        # -------- phase 2: blk-outer scan loop, y accumulated in PSUM -----
        bc = ctx.enter_context(tc.tile_pool(name="bc", bufs=1))
        Bt_t = [bc.tile([128, L], BF16, tag=f"Bt{n}", name=f"Bt{n}")
                for n in range(N_STATE)]
        Ct_t = [bc.tile([128, L], BF16, tag=f"Ct{n}", name=f"Ct{n}")
                for n in range(N_STATE)]

        def bcast(n):
            for src_r, dst in ((B_sb, Bt_t[n]), (C_sb, Ct_t[n])):
                for nch in range(NCH):
                    ps = pio.tile([128, 512], F32, tag="pio", name="ps_bc")
                    nc.tensor.matmul(
                        ps[:], lhsT=bcsel[:, n * 128:(n + 1) * 128],
                        rhs=src_r[:, nch * 512:(nch + 1) * 512],
                        start=True, stop=True)
                    nc.scalar.copy(dst[:, nch * 512:(nch + 1) * 512], ps[:])

        psy = ctx.enter_context(tc.tile_pool(name="psy", bufs=1, space="PSUM"))
        ph2 = ctx.enter_context(tc.tile_pool(name="ph2", bufs=1))
        pda = ctx.enter_context(tc.tile_pool(name="pda", bufs=2))

        bcast(0)
        bcast(1)
        bcast(2)

        for blk in range(NBLK_H):
            # recompute dt = softplus(dt_raw + dt_b) for this blk (SBUF
            # is too tight to hold all four dt tiles through phase 2)
            dte = ph2.tile([128, L], BF16, tag="dtx", name="dte")
            for nch in range(NCH):
                ps = pio.tile([128, 512], F32, tag="pio", name="ps_dt")
                nc.tensor.matmul(
                    ps[:], lhsT=dt_w[:, blk * 128:(blk + 1) * 128],
                    rhs=dbc[0:DT_RANK, nch * 512:(nch + 1) * 512],
                    start=True, stop=True)
                nc.scalar.activation(dte[:, nch * 512:(nch + 1) * 512],
                                     ps[:], AF.Exp, bias=dt_b[:, blk:blk + 1])
            dt_c = ph2.tile([128, L], BF16, tag="dt", name="dt")
            nc.scalar.activation(dt_c[:], dte[:], AF.Ln, bias=1.0)
            dtx = ph2.tile([128, L], BF16, tag="dtx", name="dtx")
            nc.vector.tensor_tensor(dtx[:], dt_c[:], xc_t[blk][:], OP.mult)
            ypsum = psy.tile([128, L], F32, tag="ypsum", name="ypsum")
            for n in range(N_STATE):
                dA = pda.tile([128, L], BF16, tag="dA", name="dA")
                nc.scalar.activation(
                    dA[:], dt_c[:], AF.Exp,
                    scale=A_sb[:, blk * N_STATE + n:blk * N_STATE + n + 1])
                dBx = ph2.tile([128, L], BF16, tag="dBx", name="dBx")
                nc.vector.tensor_tensor(dBx[:], dtx[:], Bt_t[n][:], OP.mult)
                # scan overwrites its dA input (SBUF pressure)
                nc.vector.tensor_tensor_scan(
                    dA[:], dA[:], dBx[:], 0.0, OP.mult, OP.add)
                p = ph2.tile([128, L], BF16, tag="dBx", name="p")
                nc.vector.tensor_tensor(p[:], dA[:], Ct_t[n][:], OP.mult)
                for nch in range(NCH):
                    nc.tensor.matmul(
                        ypsum[:, nch * 512:(nch + 1) * 512],
                        lhsT=ident[:], rhs=p[:, nch * 512:(nch + 1) * 512],
                        start=(n == 0), stop=False)
                if blk == 0 and n + 3 < N_STATE:
                    bcast(n + 3)
            # D skip: y += D*xc via one more PE accumulation
            dxc = pda.tile([128, L], BF16, tag="dA", name="dxc")
            nc.scalar.mul(dxc[:], xc_t[blk][:], D_sb[:, blk:blk + 1])
            for nch in range(NCH):
                nc.tensor.matmul(
                    ypsum[:, nch * 512:(nch + 1) * 512],
                    lhsT=ident[:], rhs=dxc[:, nch * 512:(nch + 1) * 512],
                    start=False, stop=True)
            # gate: t = y * silu(z); overwrites xc (dead after dxc)
            nc.vector.tensor_tensor(xc_t[blk][:], ypsum[:], gz_t[blk][:],
                                    OP.mult)

        # ---------------- phase 3: out-proj tail ----------------
        w_out_v = w_out[:].rearrange("p (k m) -> p k m", k=4)
        with tc.tile_pool(name="ph3b", bufs=2) as ph3b:
            for m in range(4):
                for nch in range(NCH):
                    ps = pio.tile([128, 512], F32, tag="pio", name="ps_out")
                    for kb in range(NBLK_H):
                        nc.tensor.matmul(
                            ps[:], lhsT=w_out_v[:, kb, m * 128:(m + 1) * 128],
                            rhs=xc_t[kb][:, nch * 512:(nch + 1) * 512],
                            start=(kb == 0), stop=(kb == NBLK_H - 1))
                    ob = ph3b.tile([128, 512], F32, tag="outb", name="outb")
                    nc.scalar.copy(ob[:], ps[:])
                    nc.sync.dma_start(
                        out_d[m * 128:(m + 1) * 128,
                              nch * 512:(nch + 1) * 512], ob[:])

    _split_excess_waits(nc)
    return nc


def _split_excess_waits(nc, max_waits=1):
    """The walrus build rejects instructions carrying more than one
    sync-wait command ("Too many sync wait commands" on Tile's kernel-tail
    Drain, which waits on every loose semaphore). Move excess waits onto
    NoOps placed just before the offender on the same engine."""
    for fn in nc.m.functions:
        for blk in fn.blocks:
            out, changed = [], False
            for inst in blk.instructions:
                si = inst.sync_info
                waits = list(si.on_wait) if si is not None and si.on_wait else []
                if len(waits) > max_waits:
                    extra, keep = waits[:-max_waits], waits[-max_waits:]
                    chunks = [extra[i:i + max_waits]
                              for i in range(0, len(extra), max_waits)]
                    for j, ch in enumerate(chunks):
                        nop = mybir.InstNoOp(
                            name=f"{inst.name}-waitsplit{j}", ins=[], outs=[])
                        nop.engine = inst.engine
                        nop.sync_info = mybir.SyncInfo(on_wait=ch, on_update=[])
                        out.append(nop)
                    si.on_wait = keep
                    changed = True
                out.append(inst)
            if changed:
                blk.instructions = out
    return nc


_PROG = None


def _get_program():
    global _PROG
    if _PROG is None:
        _PROG = _build_program()
    return _PROG


def _to_pblocks(a, nblk, dtype):
    """[nblk*128, f] -> [128, nblk*f] with [p, blk*f+j] = a[blk*128+p, j]."""
    a = np.ascontiguousarray(a)
    f = a.shape[1] if a.ndim > 1 else 1
    a = a.reshape(nblk, 128, f).transpose(1, 0, 2).reshape(128, nblk * f)
    return np.ascontiguousarray(a.astype(dtype))


def _core_inputs(hs, params, fuse_w, b, dr, h):
    p = params[dr]
    x = hs[b]
    if dr == 1:
        x = x[::-1]
    xTp = np.concatenate(
        [np.zeros((D_MODEL, 3), np.float32), np.ascontiguousarray(x.T)], axis=1)
    xT = _to_pblocks(xTp, 4, BF16NP)  # [128, 4*(L+3)] bf16

    sl_own = slice(h * DH, (h + 1) * DH)
    perm = np.r_[h * DH:(h + 1) * DH, (1 - h) * DH:(2 - h) * DH]

    in_w = p["in_w"]
    w_in_cols = np.concatenate(
        [in_w[:, :D_INNER][:, perm], in_w[:, D_INNER:][:, sl_own]], axis=1)
    w_in = _to_pblocks(w_in_cols, 4, BF16NP)

    conv_w = _to_pblocks(p["conv_w"][perm], NBLK_F, np.float32)
    conv_b = _to_pblocks(p["conv_b"][perm][:, None], NBLK_F, np.float32)
    xproj_w = _to_pblocks(p["xproj_w"][perm], NBLK_F, BF16NP)
    dt_w = np.ascontiguousarray(p["dt_w"][:, sl_own].astype(BF16NP))
    dt_b = _to_pblocks(p["dt_b"][sl_own][:, None], NBLK_H, np.float32)
    A = _to_pblocks(-np.exp(p["A_log"][sl_own]), NBLK_H, np.float32)
    D = _to_pblocks(p["D_skip"][sl_own][:, None], NBLK_H, np.float32)

    fuse_half = fuse_w[:D_MODEL] if dr == 0 else fuse_w[D_MODEL:]
    w_out_full = p["out_w"].astype(np.float64) @ fuse_half.astype(np.float64)
    w_out = _to_pblocks(w_out_full[sl_own].astype(np.float32), 4, BF16NP)

    bcsel = np.zeros((N_STATE, N_STATE * 128), BF16NP)
    for n in range(N_STATE):
        bcsel[n, n * 128:(n + 1) * 128] = 1.0
    ident = np.eye(128, dtype=BF16NP)

    return {
        "xT": xT, "w_in": w_in, "conv_w": conv_w, "conv_b": conv_b,
        "xproj_w": xproj_w, "dt_w": dt_w, "dt_b": dt_b, "A": A, "D": D,
        "w_out": w_out, "bcsel": bcsel, "ident": ident,
    }


def kernel(_spmd_kwargs=None, **inputs):
    hs = np.asarray(inputs["hidden_states"], dtype=np.float32)
    fuse_w = np.asarray(inputs["fuse_w"], dtype=np.float32)
    fuse_b = np.asarray(inputs["fuse_b"], dtype=np.float32)
    params = []
    for pre in ("fwd_", "bwd_"):
        params.append({k[len(pre):]: np.asarray(v, dtype=np.float32)
                       for k, v in inputs.items() if k.startswith(pre)})

    nc = _get_program()

    in_maps = []
    core_cfg = []
    prep_cache = {}
    for c in range(8):
        b, dr, h = c >> 2, (c >> 1) & 1, c & 1
        core_cfg.append((b, dr, h))
        key = (b, dr, h)
        if key not in prep_cache:
            prep_cache[key] = _core_inputs(hs, params, fuse_w, b, dr, h)
        in_maps.append(prep_cache[key])

    res = run_bass_kernel_spmd(nc, in_maps, core_ids=list(range(8)),
                               **(_spmd_kwargs or {}))

    out = np.zeros((B, L, D_MODEL), dtype=np.float32)
    for c in range(8):
        b, dr, h = core_cfg[c]
        contrib = res.results[c]["out_part"].T  # (L, D_MODEL)
        if dr == 1:
            contrib = contrib[::-1]
        out[b] += contrib
    out += fuse_b[None, None, :]
    if _spmd_kwargs is not None:
        kernel._last_result = res
    return out


# revision 19
# speedup vs baseline: 1.1208x; 1.1208x over previous
"""BiMambaBlock on 8 Trainium2 NeuronCores.

Sharding: core c = (batch b, direction d, d_inner-half h) with
b = c>>2, d = (c>>1)&1, h = c&1.  Every core runs the same program on
different data (weights sliced/permuted per core on the host):

  - host feeds x[b].T in bf16 (flipped along L for bwd cores, padded
    with 3 leading zero cols for the causal conv), so the device always
    runs a *forward* mamba mixer in channels-on-partitions layout [d, L].
  - each core computes the full xc = silu(conv(x @ in_w_xi)) over all
    1024 channels (so the xproj contraction over d_inner stays local,
    no collectives), but scans only its 512-channel half (the host
    permutes weights so the own half is always channel blocks 0-3).
  - out_proj and the final fuse matmul are folded on the host into one
    [512ch, 512dm] weight; each core emits a partial [512dm, L] f32
    which the host transposes/flips/sums.

v2 structure (DVE is the bottleneck: 64 tensor_tensor_scans at
~2.16 ns/elem = 283us/core are irreducible; everything else must
overlap under them or leave DVE):
  - phase 2 loops blk-outer / n-inner; y = sum_n h_n*C_n accumulates in
    PSUM via identity-lhsT matmuls on the (idle) PE instead of 60 DVE
    tensor_tensor adds.
  - B_n/C_n rows are broadcast across partitions once (selector
    matmuls + ACT drains), pipelined two n ahead inside blk0's n-loop,
    and the [128,L] bf16 broadcast tiles stay resident for blks 1-3.
  - conv taps 0/2 run as one tensor_scalar + one fused
    scalar_tensor_tensor on DVE; taps 1/3 on ACT (odd byte offsets
    would drop DVE to 1x mode).
  - z-gate matmuls issue after the dt chain, so they fill PE/ACT time
    under the first scans instead of delaying them.
  - scan writes in-place over its dA input and p reuses the dBx buffer
    (SBUF is within ~2 KiB/partition of full with 32 broadcast tiles).
"""
import os
import sys

for _p in ("/opt/trn_rl_repo",):
    if os.path.isdir(_p) and _p not in sys.path:
        sys.path.insert(0, _p)

from contextlib import ExitStack

import ml_dtypes
import numpy as np

from concourse import bass, mybir, tile
from concourse.bass_utils import run_bass_kernel_spmd

F32 = mybir.dt.float32
BF16 = mybir.dt.bfloat16
AF = mybir.ActivationFunctionType
OP = mybir.AluOpType

D_MODEL = 512
D_INNER = 1024
DH = 512
N_STATE = 16
D_CONV = 4
DT_RANK = 32
B = 2
L = 2048
LP = L + 3

NBLK_F = D_INNER // 128  # 8 channel blocks for conv/xproj
NBLK_H = DH // 128       # 4 scan blocks

NCH = L // 512           # 512-wide matmul N-chunks

BF16NP = ml_dtypes.bfloat16


def _build_program():
    nc = bass.Bass(trn_type="TRN2", target_bir_lowering=False, debug=False)

    xT_d = nc.dram_tensor("xT", [128, 4 * LP], BF16, kind="ExternalInput")
    w_in_d = nc.dram_tensor("w_in", [128, 4 * 1536], BF16, kind="ExternalInput")
    conv_w_d = nc.dram_tensor("conv_w", [128, NBLK_F * D_CONV], F32, kind="ExternalInput")
    conv_b_d = nc.dram_tensor("conv_b", [128, NBLK_F], F32, kind="ExternalInput")
    xproj_w_d = nc.dram_tensor("xproj_w", [128, NBLK_F * 64], BF16, kind="ExternalInput")
    dt_w_d = nc.dram_tensor("dt_w", [DT_RANK, DH], BF16, kind="ExternalInput")
    dt_b_d = nc.dram_tensor("dt_b", [128, NBLK_H], F32, kind="ExternalInput")
    A_d = nc.dram_tensor("A", [128, NBLK_H * N_STATE], F32, kind="ExternalInput")
    D_d = nc.dram_tensor("D", [128, NBLK_H], F32, kind="ExternalInput")
    w_out_d = nc.dram_tensor("w_out", [128, 4 * D_MODEL], BF16, kind="ExternalInput")
    bcsel_d = nc.dram_tensor("bcsel", [N_STATE, N_STATE * 128], BF16,
                             kind="ExternalInput")
    ident_d = nc.dram_tensor("ident", [128, 128], BF16, kind="ExternalInput")
    out_d = nc.dram_tensor("out_part", [D_MODEL, L], F32, kind="ExternalOutput")

    with tile.TileContext(nc) as tc, ExitStack() as ctx:
        # ---------------- persistent small weights ----------------
        wp = ctx.enter_context(tc.tile_pool(name="weights", bufs=1))

        conv_w = wp.tile([128, NBLK_F * D_CONV], F32, tag="conv_w")
        conv_b = wp.tile([128, NBLK_F], F32, tag="conv_b")
        xproj_w = wp.tile([128, NBLK_F * 64], BF16, tag="xproj_w")
        dt_w = wp.tile([DT_RANK, DH], BF16, tag="dt_w")
        dt_b = wp.tile([128, NBLK_H], F32, tag="dt_b")
        A_sb = wp.tile([128, NBLK_H * N_STATE], F32, tag="A")
        D_sb = wp.tile([128, NBLK_H], F32, tag="D")
        w_out = wp.tile([128, 4 * D_MODEL], BF16, tag="w_out")
        bcsel = wp.tile([N_STATE, N_STATE * 128], BF16, tag="bcsel")
        ident = wp.tile([128, 128], BF16, tag="ident")

        # long-lived activation tiles
        glob = ctx.enter_context(tc.tile_pool(name="glob", bufs=1))
        xc_t = [glob.tile([128, L], BF16, tag=f"xc{i}", name=f"xc{i}")
                for i in range(NBLK_H)]  # own-half xc; reused as gate output
        gz_t = [glob.tile([128, L], BF16, tag=f"gz{i}", name=f"gz{i}")
                for i in range(NBLK_H)]
        B_sb = glob.tile([N_STATE, L], BF16, tag="Brows")
        C_sb = glob.tile([N_STATE, L], BF16, tag="Crows")
        dbc = glob.tile([64, L], BF16, tag="dbc")

        # PSUM pools: pio lives the whole kernel (2 banks); phase-1-only
        # banks (pio1 + pdbc, 6) close before the phase-2 y accumulator
        # (4 banks) opens.
        pio = ctx.enter_context(tc.tile_pool(name="pio", bufs=2, space="PSUM"))

        # phase-2 pools created early: the blk0 dt chain and first
        # broadcasts are issued from inside the phase-1 scope so they
        # precede the z-gate burst in every engine's in-order queue.
        ph2 = ctx.enter_context(tc.tile_pool(name="ph2", bufs=1))
        pda = ctx.enter_context(tc.tile_pool(name="pda", bufs=2))
        bc0 = ctx.enter_context(tc.tile_pool(name="bc0", bufs=1))
        NBC0 = 3
        Bt_t = [bc0.tile([128, L], BF16, tag=f"Bt{n}", name=f"Bt{n}")
                for n in range(NBC0)]
        Ct_t = [bc0.tile([128, L], BF16, tag=f"Ct{n}", name=f"Ct{n}")
                for n in range(NBC0)]

        def bcast(n):
            for src_r, dst in ((B_sb, Bt_t[n]), (C_sb, Ct_t[n])):
                for nch in range(NCH):
                    ps = pio.tile([128, 512], F32, tag="pio", name="ps_bc")
                    nc.tensor.matmul(
                        ps[:], lhsT=bcsel[:, n * 128:(n + 1) * 128],
                        rhs=src_r[:, nch * 512:(nch + 1) * 512],
                        start=True, stop=True)
                    nc.scalar.copy(dst[:, nch * 512:(nch + 1) * 512], ps[:])

        def dt_chain(blk):
            """dt = softplus(dbc_dt @ dt_w + dt_b) for one blk."""
            dte = ph2.tile([128, L], BF16, tag="dtx", name="dte")
            for nch in range(NCH):
                ps = pio.tile([128, 512], F32, tag="pio", name="ps_dt")
                nc.tensor.matmul(
                    ps[:], lhsT=dt_w[:, blk * 128:(blk + 1) * 128],
                    rhs=dbc[0:DT_RANK, nch * 512:(nch + 1) * 512],
                    start=True, stop=True)
                nc.scalar.activation(dte[:, nch * 512:(nch + 1) * 512],
                                     ps[:], AF.Exp, bias=dt_b[:, blk:blk + 1])
            dt_c = ph2.tile([128, L], BF16, tag="dt", name="dt")
            nc.scalar.activation(dt_c[:], dte[:], AF.Ln, bias=1.0)
            return dt_c

        with tc.tile_pool(name="pin", bufs=1) as pin, \
             tc.tile_pool(name="ph1b", bufs=2) as ph1b, \
             tc.tile_pool(name="ph1c", bufs=2) as ph1c, \
             tc.tile_pool(name="pio1", bufs=2, space="PSUM") as pio1, \
             tc.tile_pool(name="pdbc", bufs=1, space="PSUM") as pdbc:
            xT = pin.tile([128, 4 * LP], BF16, tag="xT")
            w_in = pin.tile([128, 4 * 1536], BF16, tag="w_in")

            # fine-grained DMA order: first matmul's operands land first
            for kb in range(4):
                nc.sync.dma_start(xT[:, kb * LP:kb * LP + 515],
                                  xT_d[:, kb * LP:kb * LP + 515])
            for kb in range(4):
                nc.sync.dma_start(w_in[:, kb * 1536:kb * 1536 + 768],
                                  w_in_d[:, kb * 1536:kb * 1536 + 768])
            for nch in range(1, 4):
                for kb in range(4):
                    o = kb * LP + 3 + nch * 512
                    nc.sync.dma_start(xT[:, o:o + 512], xT_d[:, o:o + 512])
            for kb in range(4):
                nc.sync.dma_start(w_in[:, kb * 1536 + 768:(kb + 1) * 1536],
                                  w_in_d[:, kb * 1536 + 768:(kb + 1) * 1536])
            for t, d in [(conv_w, conv_w_d),
                         (conv_b, conv_b_d), (xproj_w, xproj_w_d),
                         (dt_w, dt_w_d), (dt_b, dt_b_d), (A_sb, A_d),
                         (D_sb, D_d), (w_out, w_out_d), (bcsel, bcsel_d),
                         (ident, ident_d)]:
                nc.sync.dma_start(t[:], d[:])

            xT_v = xT[:].rearrange("p (k l) -> p k l", k=4)
            w_in_v = w_in[:].rearrange("p (k m) -> p k m", k=4)
            xproj_v = xproj_w[:].rearrange("p (k f) -> p k f", k=NBLK_F)

            # xproj accumulators persist across the whole blk loop
            ps_dbc = [pdbc.tile([64, 512], F32, tag=f"pdbc{nch}",
                                name=f"ps_dbc{nch}") for nch in range(NCH)]

            # ---------------- phase 1: xc / xproj / dt ----------------
            for blk in range(NBLK_F):
                xi = ph1b.tile([128, LP], BF16, tag="xi", name="xi")
                m0 = blk * 128
                for gi, nch in enumerate(range(0, LP, 512)):
                    w = min(512, LP - nch)
                    ps = pio1.tile([128, 512], F32, tag="pio1", name="ps_in")
                    for kb in range(4):
                        nc.tensor.matmul(
                            ps[:, 0:w],
                            lhsT=w_in_v[:, kb, m0:m0 + 128],
                            rhs=xT_v[:, kb, nch:nch + w],
                            start=(kb == 0), stop=(kb == 3),
                        )
                    nc.scalar.copy(xi[:, nch:nch + w], ps[:, 0:w])
                # conv: taps 0/2 on DVE (even offsets keep 4B alignment →
                # 4x tensor_scalar), taps 1/3 on ACT, pair-adds on DVE
                acc = ph1c.tile([128, L], BF16, tag="ct0", name="ct0")
                nc.vector.tensor_scalar_mul(
                    acc[:], xi[:, 0:L], conv_w[:, blk * 4:blk * 4 + 1])
                t2 = ph1c.tile([128, L], BF16, tag="ct2", name="ct2")
                nc.vector.tensor_scalar_mul(
                    t2[:], xi[:, 2:2 + L], conv_w[:, blk * 4 + 2:blk * 4 + 3])
                t1 = ph1c.tile([128, L], BF16, tag="ct1", name="ct1")
                nc.scalar.mul(t1[:], xi[:, 1:1 + L],
                              conv_w[:, blk * 4 + 1:blk * 4 + 2])
                t3 = ph1c.tile([128, L], BF16, tag="ct3", name="ct3")
                nc.scalar.mul(t3[:], xi[:, 3:3 + L],
                              conv_w[:, blk * 4 + 3:blk * 4 + 4])
                nc.vector.tensor_tensor(acc[:], acc[:], t2[:], OP.add)
                nc.vector.tensor_tensor(t1[:], t1[:], t3[:], OP.add)
                nc.vector.tensor_tensor(acc[:], acc[:], t1[:], OP.add)
                if blk < NBLK_H:
                    xc = xc_t[blk]
                else:
                    xc = ph1b.tile([128, L], BF16, tag="xcO", name=f"xcO{blk}")
                nc.scalar.activation(xc[:], acc[:], AF.Silu,
                                     bias=conv_b[:, blk:blk + 1])
                if blk >= NBLK_H:
                    xc_t.append(xc)

                # xproj contribution of this block (accumulates over blks)
                for nch in range(NCH):
                    nc.tensor.matmul(
                        ps_dbc[nch][:], lhsT=xproj_v[:, blk, :],
                        rhs=xc[:, nch * 512:(nch + 1) * 512],
                        start=(blk == 0), stop=(blk == NBLK_F - 1),
                    )
                    if blk == NBLK_F - 1:
                        nc.scalar.copy(dbc[:, nch * 512:(nch + 1) * 512],
                                       ps_dbc[nch][:])
                # two z-gate chunks per iteration: spreads the 16 z
                # matmul+silu chunks through phase 1 so nothing bursts at
                # the phase-1 -> phase-2 transition
                for c in (2 * blk, 2 * blk + 1):
                    zb, zn = c // NCH, c % NCH
                    ps = pio1.tile([128, 512], F32, tag="pio1", name="ps_z")
                    for kb in range(4):
                        nc.tensor.matmul(
                            ps[:],
                            lhsT=w_in_v[:, kb, 1024 + zb * 128:1024 + (zb + 1) * 128],
                            rhs=xT_v[:, kb, 3 + zn * 512:3 + (zn + 1) * 512],
                            start=(kb == 0), stop=(kb == 3),
                        )
                    nc.scalar.activation(gz_t[zb][:, zn * 512:(zn + 1) * 512],
                                         ps[:], AF.Silu)

            nc.sync.dma_start(B_sb[:], dbc[32:48, :])
            nc.sync.dma_start(C_sb[:], dbc[48:64, :])

            # transition-critical work: blk0's dt chain + first broadcasts
            dt0 = dt_chain(0)
            for n in range(NBC0):
                bcast(n)

        # -------- phase 2: blk-outer scan loop, y accumulated in PSUM -----
        bc = ctx.enter_context(tc.tile_pool(name="bc", bufs=1))
        Bt_t.extend(bc.tile([128, L], BF16, tag=f"Bt{n}", name=f"Bt{n}")
                    for n in range(NBC0, N_STATE))
        Ct_t.extend(bc.tile([128, L], BF16, tag=f"Ct{n}", name=f"Ct{n}")
                    for n in range(NBC0, N_STATE))

        psy = ctx.enter_context(tc.tile_pool(name="psy", bufs=1, space="PSUM"))

        for blk in range(NBLK_H):
            dt_c = dt0 if blk == 0 else dt_chain(blk)
            dtx = ph2.tile([128, L], BF16, tag="dtx", name="dtx")
            nc.vector.tensor_tensor(dtx[:], dt_c[:], xc_t[blk][:], OP.mult)
            ypsum = psy.tile([128, L], F32, tag="ypsum", name="ypsum")
            for n in range(N_STATE):
                dA = pda.tile([128, L], BF16, tag="dA", name="dA")
                nc.scalar.activation(
                    dA[:], dt_c[:], AF.Exp,
                    scale=A_sb[:, blk * N_STATE + n:blk * N_STATE + n + 1])
                if blk == 0 and n + NBC0 < N_STATE:
                    bcast(n + NBC0)
                dBx = ph2.tile([128, L], BF16, tag="dBx", name="dBx")
                nc.vector.tensor_tensor(dBx[:], dtx[:], Bt_t[n][:], OP.mult)
                # scan overwrites its dA input, p overwrites the scan
                # output; pda's two buffers give the PE readers slack
                nc.vector.tensor_tensor_scan(
                    dA[:], dA[:], dBx[:], 0.0, OP.mult, OP.add)
                nc.vector.tensor_tensor(dA[:], dA[:], Ct_t[n][:], OP.mult)
                for nch in range(NCH):
                    nc.tensor.matmul(
                        ypsum[:, nch * 512:(nch + 1) * 512],
                        lhsT=ident[:], rhs=dA[:, nch * 512:(nch + 1) * 512],
                        start=(n == 0), stop=False)
            # D skip: y += D*xc via one more PE accumulation
            dxc = pda.tile([128, L], BF16, tag="dA", name="dxc")
            nc.scalar.mul(dxc[:], xc_t[blk][:], D_sb[:, blk:blk + 1])
            for nch in range(NCH):
                nc.tensor.matmul(
                    ypsum[:, nch * 512:(nch + 1) * 512],
                    lhsT=ident[:], rhs=dxc[:, nch * 512:(nch + 1) * 512],
                    start=False, stop=True)
            # gate: t = y * silu(z); overwrites xc (dead after dxc)
            nc.vector.tensor_tensor(xc_t[blk][:], ypsum[:], gz_t[blk][:],
                                    OP.mult)

        # ---------------- phase 3: out-proj tail ----------------
        w_out_v = w_out[:].rearrange("p (k m) -> p k m", k=4)
        with tc.tile_pool(name="ph3b", bufs=2) as ph3b:
            for m in range(4):
                for nch in range(NCH):
                    ps = pio.tile([128, 512], F32, tag="pio", name="ps_out")
                    for kb in range(NBLK_H):
                        nc.tensor.matmul(
                            ps[:], lhsT=w_out_v[:, kb, m * 128:(m + 1) * 128],
                            rhs=xc_t[kb][:, nch * 512:(nch + 1) * 512],
                            start=(kb == 0), stop=(kb == NBLK_H - 1))
                    ob = ph3b.tile([128, 512], F32, tag="outb", name="outb")
                    nc.scalar.copy(ob[:], ps[:])
                    nc.sync.dma_start(
                        out_d[m * 128:(m + 1) * 128,
                              nch * 512:(nch + 1) * 512], ob[:])

    _split_excess_waits(nc)
    return nc


def _split_excess_waits(nc, max_waits=1):
    """The walrus build rejects instructions carrying more than one
    sync-wait command ("Too many sync wait commands" on Tile's kernel-tail
    Drain, which waits on every loose semaphore). Move excess waits onto
    NoOps placed just before the offender on the same engine."""
    for fn in nc.m.functions:
        for blk in fn.blocks:
            out, changed = [], False
            for inst in blk.instructions:
                si = inst.sync_info
                waits = list(si.on_wait) if si is not None and si.on_wait else []
                if len(waits) > max_waits:
                    extra, keep = waits[:-max_waits], waits[-max_waits:]
                    chunks = [extra[i:i + max_waits]
                              for i in range(0, len(extra), max_waits)]
                    for j, ch in enumerate(chunks):
                        nop = mybir.InstNoOp(
                            name=f"{inst.name}-waitsplit{j}", ins=[], outs=[])
                        nop.engine = inst.engine
                        nop.sync_info = mybir.SyncInfo(on_wait=ch, on_update=[])
                        out.append(nop)
                    si.on_wait = keep
                    changed = True
                out.append(inst)
            if changed:
                blk.instructions = out
    return nc


_PROG = None


def _get_program():
    global _PROG
    if _PROG is None:
        _PROG = _build_program()
    return _PROG


def _to_pblocks(a, nblk, dtype):
    """[nblk*128, f] -> [128, nblk*f] with [p, blk*f+j] = a[blk*128+p, j]."""
    a = np.ascontiguousarray(a)
    f = a.shape[1] if a.ndim > 1 else 1
    a = a.reshape(nblk, 128, f).transpose(1, 0, 2).reshape(128, nblk * f)
    return np.ascontiguousarray(a.astype(dtype))


def _core_inputs(hs, params, fuse_w, b, dr, h):
    p = params[dr]
    x = hs[b]
    if dr == 1:
        x = x[::-1]
    xTp = np.concatenate(
        [np.zeros((D_MODEL, 3), np.float32), np.ascontiguousarray(x.T)], axis=1)
    xT = _to_pblocks(xTp, 4, BF16NP)  # [128, 4*(L+3)] bf16

    sl_own = slice(h * DH, (h + 1) * DH)
    perm = np.r_[h * DH:(h + 1) * DH, (1 - h) * DH:(2 - h) * DH]

    in_w = p["in_w"]
    w_in_cols = np.concatenate(
        [in_w[:, :D_INNER][:, perm], in_w[:, D_INNER:][:, sl_own]], axis=1)
    w_in = _to_pblocks(w_in_cols, 4, BF16NP)

    conv_w = _to_pblocks(p["conv_w"][perm], NBLK_F, np.float32)
    conv_b = _to_pblocks(p["conv_b"][perm][:, None], NBLK_F, np.float32)
    xproj_w = _to_pblocks(p["xproj_w"][perm], NBLK_F, BF16NP)
    dt_w = np.ascontiguousarray(p["dt_w"][:, sl_own].astype(BF16NP))
    dt_b = _to_pblocks(p["dt_b"][sl_own][:, None], NBLK_H, np.float32)
    A = _to_pblocks(-np.exp(p["A_log"][sl_own]), NBLK_H, np.float32)
    D = _to_pblocks(p["D_skip"][sl_own][:, None], NBLK_H, np.float32)

    fuse_half = fuse_w[:D_MODEL] if dr == 0 else fuse_w[D_MODEL:]
    w_out_full = p["out_w"].astype(np.float64) @ fuse_half.astype(np.float64)
    w_out = _to_pblocks(w_out_full[sl_own].astype(np.float32), 4, BF16NP)

    bcsel = np.zeros((N_STATE, N_STATE * 128), BF16NP)
    for n in range(N_STATE):
        bcsel[n, n * 128:(n + 1) * 128] = 1.0
    ident = np.eye(128, dtype=BF16NP)

    return {
        "xT": xT, "w_in": w_in, "conv_w": conv_w, "conv_b": conv_b,
        "xproj_w": xproj_w, "dt_w": dt_w, "dt_b": dt_b, "A": A, "D": D,
        "w_out": w_out, "bcsel": bcsel, "ident": ident,
    }


def kernel(_spmd_kwargs=None, **inputs):
    hs = np.asarray(inputs["hidden_states"], dtype=np.float32)
    fuse_w = np.asarray(inputs["fuse_w"], dtype=np.float32)
    fuse_b = np.asarray(inputs["fuse_b"], dtype=np.float32)
    params = []
    for pre in ("fwd_", "bwd_"):
        params.append({k[len(pre):]: np.asarray(v, dtype=np.float32)
                       for k, v in inputs.items() if k.startswith(pre)})

    nc = _get_program()

    in_maps = []
    core_cfg = []
    prep_cache = {}
    for c in range(8):
        b, dr, h = c >> 2, (c >> 1) & 1, c & 1
        core_cfg.append((b, dr, h))
        key = (b, dr, h)
        if key not in prep_cache:
            prep_cache[key] = _core_inputs(hs, params, fuse_w, b, dr, h)
        in_maps.append(prep_cache[key])

    res = run_bass_kernel_spmd(nc, in_maps, core_ids=list(range(8)),
                               **(_spmd_kwargs or {}))

    out = np.zeros((B, L, D_MODEL), dtype=np.float32)
    for c in range(8):
        b, dr, h = core_cfg[c]
        contrib = res.results[c]["out_part"].T  # (L, D_MODEL)
        if dr == 1:
            contrib = contrib[::-1]
        out[b] += contrib
    out += fuse_b[None, None, :]
    if _spmd_kwargs is not None:
        kernel._last_result = res
    return out


# revision 21
# speedup vs baseline: 1.2450x; 1.1108x over previous
"""BiMambaBlock on 8 Trainium2 NeuronCores.

Sharding: core c = (batch b, direction d, d_inner-half h) with
b = c>>2, d = (c>>1)&1, h = c&1.  Every core runs the same program on
different data (weights sliced/permuted per core on the host):

  - host feeds x[b].T in bf16 (flipped along L for bwd cores, padded
    with 4 leading zero cols for the causal conv), so the device always
    runs a *forward* mamba mixer in channels-on-partitions layout [d, L].
  - each core computes the full xc = silu(conv(x @ in_w_xi)) over all
    1024 channels (so the xproj contraction over d_inner stays local,
    no collectives), but scans only its 512-channel half (the host
    permutes weights so the own half is always channel blocks 0-3).
  - out_proj and the final fuse matmul are folded on the host into one
    [512ch, 512dm] weight; each core emits a partial [512dm, L] f32
    which the host transposes/flips/sums.

v3 structure.  The 64 DVE tensor_tensor_scans (4.42us each, 283us)
are the hard floor; everything else is arranged to overlap under them:
  - B_n/C_n rows round-trip through a DRAM scratch and are broadcast
    across partitions by stride-0-source DMAs (measured exact): no PE
    selector matmuls, no ACT drains, so every blk runs at DVE pace.
    C tiles (16) stay resident; B tiles rotate through 4 buffers with
    3-ahead prefetch.
  - phase 2 loops blk-outer / n-inner; y accumulates in PSUM via
    identity-lhsT matmuls on the otherwise idle PE (replaces 60 DVE
    adds); scan writes in-place over dA and the C-mul in-place over the
    scan output, with 2 dA buffers giving the PE reader slack.
  - the x padding is 4 columns so xi[:, 0:4] is exactly zero: the
    in_proj tail chunk (3 cols = 32 matmuls) becomes one memset, and
    the even conv-tap offsets stay 4B-aligned for DVE 4x tensor_scalar.
  - the z gate runs inside each blk's n-loop (xT persists; z weights
    are a separate small input), keeping all 64 z matmuls off the
    serial phase-1 prefix.
  - GPSIMD stays idle on purpose: measured tensor_tensor there is 5.9x
    slower AND degrades concurrent DVE scans ~50% via the shared port.
"""
import os
import sys

for _p in ("/opt/trn_rl_repo",):
    if os.path.isdir(_p) and _p not in sys.path:
        sys.path.insert(0, _p)

from contextlib import ExitStack

import ml_dtypes
import numpy as np

from concourse import bass, mybir, tile
from concourse.bass_utils import run_bass_kernel_spmd

F32 = mybir.dt.float32
BF16 = mybir.dt.bfloat16
AF = mybir.ActivationFunctionType
OP = mybir.AluOpType

D_MODEL = 512
D_INNER = 1024
DH = 512
N_STATE = 16
D_CONV = 4
DT_RANK = 32
B = 2
L = 2048
PAD = 4
LP = L + PAD

NBLK_F = D_INNER // 128  # 8 channel blocks for conv/xproj
NBLK_H = DH // 128       # 4 scan blocks

NCH = L // 512           # 512-wide matmul N-chunks

BF16NP = ml_dtypes.bfloat16


def _build_program():
    nc = bass.Bass(trn_type="TRN2", target_bir_lowering=False, debug=False)

    xT_d = nc.dram_tensor("xT", [128, 4 * LP], BF16, kind="ExternalInput")
    w_in_d = nc.dram_tensor("w_in", [128, 4 * 1024], BF16, kind="ExternalInput")
    w_z_d = nc.dram_tensor("w_z", [128, 4 * 512], BF16, kind="ExternalInput")
    conv_w_d = nc.dram_tensor("conv_w", [128, NBLK_F * D_CONV], F32, kind="ExternalInput")
    conv_b_d = nc.dram_tensor("conv_b", [128, NBLK_F], F32, kind="ExternalInput")
    xproj_w_d = nc.dram_tensor("xproj_w", [128, NBLK_F * 64], BF16, kind="ExternalInput")
    dt_w_d = nc.dram_tensor("dt_w", [DT_RANK, DH], BF16, kind="ExternalInput")
    dt_b_d = nc.dram_tensor("dt_b", [128, NBLK_H], F32, kind="ExternalInput")
    A_d = nc.dram_tensor("A", [128, NBLK_H * N_STATE], F32, kind="ExternalInput")
    D_d = nc.dram_tensor("D", [128, NBLK_H], F32, kind="ExternalInput")
    w_out_d = nc.dram_tensor("w_out", [128, 4 * D_MODEL], BF16, kind="ExternalInput")
    ident_d = nc.dram_tensor("ident", [128, 128], BF16, kind="ExternalInput")
    bcrows_d = nc.dram_tensor("bc_rows", [32, L], BF16, kind="ExternalOutput")
    out_d = nc.dram_tensor("out_part", [D_MODEL, L], F32, kind="ExternalOutput")

    with tile.TileContext(nc) as tc, ExitStack() as ctx:
        # ---------------- persistent tiles ----------------
        wp = ctx.enter_context(tc.tile_pool(name="weights", bufs=1))
        xT = wp.tile([128, 4 * LP], BF16, tag="xT")
        w_z = wp.tile([128, 4 * 512], BF16, tag="w_z")
        conv_w = wp.tile([128, NBLK_F * D_CONV], F32, tag="conv_w")
        conv_b = wp.tile([128, NBLK_F], F32, tag="conv_b")
        xproj_w = wp.tile([128, NBLK_F * 64], BF16, tag="xproj_w")
        dt_w = wp.tile([DT_RANK, DH], BF16, tag="dt_w")
        dt_b = wp.tile([128, NBLK_H], F32, tag="dt_b")
        A_sb = wp.tile([128, NBLK_H * N_STATE], F32, tag="A")
        D_sb = wp.tile([128, NBLK_H], F32, tag="D")
        w_out = wp.tile([128, 4 * D_MODEL], BF16, tag="w_out")
        ident = wp.tile([128, 128], BF16, tag="ident")

        glob = ctx.enter_context(tc.tile_pool(name="glob", bufs=1))
        xc_t = [glob.tile([128, L], BF16, tag=f"xc{i}", name=f"xc{i}")
                for i in range(NBLK_H)]  # own-half xc; reused as gate output
        dt_t = [glob.tile([128, L], BF16, tag=f"dt{i}", name=f"dt{i}")
                for i in range(NBLK_H)]
        gz = glob.tile([128, L], BF16, tag="gz")
        dbc = glob.tile([64, L], BF16, tag="dbc")

        pio = ctx.enter_context(tc.tile_pool(name="pio", bufs=2, space="PSUM"))
        ph2 = ctx.enter_context(tc.tile_pool(name="ph2", bufs=1))
        pdtx = ctx.enter_context(tc.tile_pool(name="pdtx", bufs=2))
        pda = ctx.enter_context(tc.tile_pool(name="pda", bufs=2))

        xT_v = xT[:].rearrange("p (k l) -> p k l", k=4)
        w_z_v = w_z[:].rearrange("p (k m) -> p k m", k=4)
        xproj_v = xproj_w[:].rearrange("p (k f) -> p k f", k=NBLK_F)

        def dt_chain(blk):
            """dt_t[blk] = softplus(dbc_dt @ dt_w + dt_b)."""
            dte = ph2.tile([128, L], BF16, tag="dte", name="dte")
            for nch in range(NCH):
                ps = pio.tile([128, 512], F32, tag="pio", name="ps_dt")
                nc.tensor.matmul(
                    ps[:], lhsT=dt_w[:, blk * 128:(blk + 1) * 128],
                    rhs=dbc[0:DT_RANK, nch * 512:(nch + 1) * 512],
                    start=True, stop=True)
                nc.scalar.activation(dte[:, nch * 512:(nch + 1) * 512],
                                     ps[:], AF.Exp, bias=dt_b[:, blk:blk + 1])
            nc.scalar.activation(dt_t[blk][:], dte[:], AF.Ln, bias=1.0)

        # ---------------- phase 1: xc / xproj ----------------
        with tc.tile_pool(name="pin", bufs=1) as pin, \
             tc.tile_pool(name="ph1b", bufs=2) as ph1b, \
             tc.tile_pool(name="ph1c", bufs=2) as ph1c, \
             tc.tile_pool(name="pio1", bufs=2, space="PSUM") as pio1, \
             tc.tile_pool(name="pdbc", bufs=1, space="PSUM") as pdbc:
            w_in = pin.tile([128, 4 * 1024], BF16, tag="w_in")

            # fine-grained DMA order: first matmul's operands land first
            for kb in range(4):
                nc.sync.dma_start(xT[:, kb * LP:kb * LP + PAD + 512],
                                  xT_d[:, kb * LP:kb * LP + PAD + 512])
            for kb in range(4):
                nc.sync.dma_start(w_in[:, kb * 1024:kb * 1024 + 512],
                                  w_in_d[:, kb * 1024:kb * 1024 + 512])
            for nch in range(1, 4):
                for kb in range(4):
                    o = kb * LP + PAD + nch * 512
                    nc.sync.dma_start(xT[:, o:o + 512], xT_d[:, o:o + 512])
            for kb in range(4):
                nc.sync.dma_start(w_in[:, kb * 1024 + 512:(kb + 1) * 1024],
                                  w_in_d[:, kb * 1024 + 512:(kb + 1) * 1024])
            for t, d in [(w_z, w_z_d), (conv_w, conv_w_d),
                         (conv_b, conv_b_d), (xproj_w, xproj_w_d),
                         (dt_w, dt_w_d), (dt_b, dt_b_d), (A_sb, A_d),
                         (D_sb, D_d), (w_out, w_out_d), (ident, ident_d)]:
                nc.sync.dma_start(t[:], d[:])

            w_in_v = w_in[:].rearrange("p (k m) -> p k m", k=4)
            ps_dbc = [pdbc.tile([64, 512], F32, tag=f"pdbc{nch}",
                                name=f"ps_dbc{nch}") for nch in range(NCH)]

            for blk in range(NBLK_F):
                xi = ph1b.tile([128, LP], BF16, tag="xi", name="xi")
                nc.vector.memset(xi[:, 0:PAD], 0.0)
                m0 = blk * 128
                for nch in range(NCH):
                    ps = pio1.tile([128, 512], F32, tag="pio1", name="ps_in")
                    for kb in range(4):
                        nc.tensor.matmul(
                            ps[:],
                            lhsT=w_in_v[:, kb, m0:m0 + 128],
                            rhs=xT_v[:, kb, PAD + nch * 512:PAD + (nch + 1) * 512],
                            start=(kb == 0), stop=(kb == 3),
                        )
                    nc.scalar.copy(xi[:, PAD + nch * 512:PAD + (nch + 1) * 512],
                                   ps[:])
                # conv: tap offsets 2/4 on DVE (4B-aligned -> 4x
                # tensor_scalar), offsets 1/3 on ACT, pair-adds on DVE.
                # offset k+1 carries conv weight k.
                acc = ph1c.tile([128, L], BF16, tag="ct0", name="ct0")
                nc.vector.tensor_scalar_mul(
                    acc[:], xi[:, 2:2 + L], conv_w[:, blk * 4 + 1:blk * 4 + 2])
                t2 = ph1c.tile([128, L], BF16, tag="ct2", name="ct2")
                nc.vector.tensor_scalar_mul(
                    t2[:], xi[:, 4:4 + L], conv_w[:, blk * 4 + 3:blk * 4 + 4])
                t1 = ph1c.tile([128, L], BF16, tag="ct1", name="ct1")
                nc.scalar.mul(t1[:], xi[:, 1:1 + L],
                              conv_w[:, blk * 4:blk * 4 + 1])
                t3 = ph1c.tile([128, L], BF16, tag="ct3", name="ct3")
                nc.scalar.mul(t3[:], xi[:, 3:3 + L],
                              conv_w[:, blk * 4 + 2:blk * 4 + 3])
                nc.vector.tensor_tensor(acc[:], acc[:], t2[:], OP.add)
                nc.vector.tensor_tensor(t1[:], t1[:], t3[:], OP.add)
                nc.vector.tensor_tensor(acc[:], acc[:], t1[:], OP.add)
                if blk < NBLK_H:
                    xc = xc_t[blk]
                else:
                    xc = ph1b.tile([128, L], BF16, tag="xcO", name=f"xcO{blk}")
                nc.scalar.activation(xc[:], acc[:], AF.Silu,
                                     bias=conv_b[:, blk:blk + 1])
                # xproj contribution (accumulates over all 8 blocks)
                for nch in range(NCH):
                    nc.tensor.matmul(
                        ps_dbc[nch][:], lhsT=xproj_v[:, blk, :],
                        rhs=xc[:, nch * 512:(nch + 1) * 512],
                        start=(blk == 0), stop=(blk == NBLK_F - 1),
                    )
                    if blk == NBLK_F - 1:
                        nc.scalar.copy(dbc[:, nch * 512:(nch + 1) * 512],
                                       ps_dbc[nch][:])

        # ------- transition: B/C rows -> DRAM, first broadcasts, dt0 -------
        nc.sync.dma_start(bcrows_d[:], dbc[32:64, :])

        pct = ctx.enter_context(tc.tile_pool(name="pct", bufs=1))
        Ct_t = [pct.tile([128, L], BF16, tag=f"Ct{n}", name=f"Ct{n}")
                for n in range(N_STATE)]
        pbt = ctx.enter_context(tc.tile_pool(name="pbt", bufs=4))
        bt_slot = {}

        def fetch_B(g):
            """stride-0 DMA broadcast of B row (g = blk*16 + n)."""
            t = pbt.tile([128, L], BF16, tag="Bt", name=f"Bt{g}")
            nc.sync.dma_start(
                t[:], bcrows_d[g % 16:g % 16 + 1, :].to_broadcast([128, L]))
            bt_slot[g] = t

        def fetch_C(n):
            nc.sync.dma_start(
                Ct_t[n][:],
                bcrows_d[16 + n:16 + n + 1, :].to_broadcast([128, L]))

        for g in range(3):
            fetch_B(g)
            fetch_C(g)
        dt_chain(0)

        psy = ctx.enter_context(tc.tile_pool(name="psy", bufs=1, space="PSUM"))

        # -------- phase 2: blk-outer scan loop, y accumulated in PSUM -----
        for blk in range(NBLK_H):
            dtx = pdtx.tile([128, L], BF16, tag="dtx", name="dtx")
            nc.vector.tensor_tensor(dtx[:], dt_t[blk][:], xc_t[blk][:],
                                    OP.mult)
            ypsum = psy.tile([128, L], F32, tag="ypsum", name="ypsum")
            for n in range(N_STATE):
                g = blk * N_STATE + n
                dA = pda.tile([128, L], BF16, tag="dA", name="dA")
                nc.scalar.activation(
                    dA[:], dt_t[blk][:], AF.Exp,
                    scale=A_sb[:, blk * N_STATE + n:blk * N_STATE + n + 1])
                if g + 3 < NBLK_H * N_STATE:
                    fetch_B(g + 3)
                if blk == 0 and n + 3 < N_STATE:
                    fetch_C(n + 3)
                dBx = ph2.tile([128, L], BF16, tag="dBx", name="dBx")
                nc.vector.tensor_tensor(dBx[:], dtx[:], bt_slot.pop(g)[:],
                                        OP.mult)
                # scan overwrites dA; the C-mul overwrites the scan output
                nc.vector.tensor_tensor_scan(
                    dA[:], dA[:], dBx[:], 0.0, OP.mult, OP.add)
                nc.vector.tensor_tensor(dA[:], dA[:], Ct_t[n][:], OP.mult)
                for nch in range(NCH):
                    nc.tensor.matmul(
                        ypsum[:, nch * 512:(nch + 1) * 512],
                        lhsT=ident[:], rhs=dA[:, nch * 512:(nch + 1) * 512],
                        start=(n == 0), stop=False)
                # z-gate chunks and the next blk's dt chain ride inside
                # the n-loop: PE and ACT have slack under the scans
                if n in (2, 5, 8, 11):
                    zn = (2, 5, 8, 11).index(n)
                    ps = pio.tile([128, 512], F32, tag="pio", name="ps_z")
                    for kb in range(4):
                        nc.tensor.matmul(
                            ps[:],
                            lhsT=w_z_v[:, kb, blk * 128:(blk + 1) * 128],
                            rhs=xT_v[:, kb, PAD + zn * 512:PAD + (zn + 1) * 512],
                            start=(kb == 0), stop=(kb == 3),
                        )
                    nc.scalar.activation(gz[:, zn * 512:(zn + 1) * 512],
                                         ps[:], AF.Silu)
                if n == 10 and blk + 1 < NBLK_H:
                    dt_chain(blk + 1)
            # D skip: y += D*xc via one more PE accumulation
            dxc = pda.tile([128, L], BF16, tag="dA", name="dxc")
            nc.scalar.mul(dxc[:], xc_t[blk][:], D_sb[:, blk:blk + 1])
            for nch in range(NCH):
                nc.tensor.matmul(
                    ypsum[:, nch * 512:(nch + 1) * 512],
                    lhsT=ident[:], rhs=dxc[:, nch * 512:(nch + 1) * 512],
                    start=False, stop=True)
            # gate: t = y * silu(z); overwrites xc (dead after dxc)
            nc.vector.tensor_tensor(xc_t[blk][:], ypsum[:], gz[:], OP.mult)

        # ---------------- phase 3: out-proj tail ----------------
        w_out_v = w_out[:].rearrange("p (k m) -> p k m", k=4)
        with tc.tile_pool(name="ph3b", bufs=2) as ph3b:
            for m in range(4):
                for nch in range(NCH):
                    ps = pio.tile([128, 512], F32, tag="pio", name="ps_out")
                    for kb in range(NBLK_H):
                        nc.tensor.matmul(
                            ps[:], lhsT=w_out_v[:, kb, m * 128:(m + 1) * 128],
                            rhs=xc_t[kb][:, nch * 512:(nch + 1) * 512],
                            start=(kb == 0), stop=(kb == NBLK_H - 1))
                    ob = ph3b.tile([128, 512], F32, tag="outb", name="outb")
                    nc.scalar.copy(ob[:], ps[:])
                    nc.sync.dma_start(
                        out_d[m * 128:(m + 1) * 128,
                              nch * 512:(nch + 1) * 512], ob[:])

    _split_excess_waits(nc)
    return nc


def _split_excess_waits(nc, max_waits=1):
    """The walrus build rejects instructions carrying more than one
    sync-wait command ("Too many sync wait commands" on Tile's kernel-tail
    Drain, which waits on every loose semaphore). Move excess waits onto
    NoOps placed just before the offender on the same engine."""
    for fn in nc.m.functions:
        for blk in fn.blocks:
            out, changed = [], False
            for inst in blk.instructions:
                si = inst.sync_info
                waits = list(si.on_wait) if si is not None and si.on_wait else []
                if len(waits) > max_waits:
                    extra, keep = waits[:-max_waits], waits[-max_waits:]
                    chunks = [extra[i:i + max_waits]
                              for i in range(0, len(extra), max_waits)]
                    for j, ch in enumerate(chunks):
                        nop = mybir.InstNoOp(
                            name=f"{inst.name}-waitsplit{j}", ins=[], outs=[])
                        nop.engine = inst.engine
                        nop.sync_info = mybir.SyncInfo(on_wait=ch, on_update=[])
                        out.append(nop)
                    si.on_wait = keep
                    changed = True
                out.append(inst)
            if changed:
                blk.instructions = out
    return nc


_PROG = None


def _get_program():
    global _PROG
    if _PROG is None:
        _PROG = _build_program()
    return _PROG


def _to_pblocks(a, nblk, dtype):
    """[nblk*128, f] -> [128, nblk*f] with [p, blk*f+j] = a[blk*128+p, j]."""
    a = np.ascontiguousarray(a)
    f = a.shape[1] if a.ndim > 1 else 1
    a = a.reshape(nblk, 128, f).transpose(1, 0, 2).reshape(128, nblk * f)
    return np.ascontiguousarray(a.astype(dtype))


def _core_inputs(hs, params, fuse_w, b, dr, h):
    p = params[dr]
    x = hs[b]
    if dr == 1:
        x = x[::-1]
    xTp = np.concatenate(
        [np.zeros((D_MODEL, PAD), np.float32), np.ascontiguousarray(x.T)],
        axis=1)
    xT = _to_pblocks(xTp, 4, BF16NP)  # [128, 4*(L+PAD)] bf16

    sl_own = slice(h * DH, (h + 1) * DH)
    perm = np.r_[h * DH:(h + 1) * DH, (1 - h) * DH:(2 - h) * DH]

    in_w = p["in_w"]
    w_in = _to_pblocks(in_w[:, :D_INNER][:, perm], 4, BF16NP)
    w_z = _to_pblocks(in_w[:, D_INNER:][:, sl_own], 4, BF16NP)

    conv_w = _to_pblocks(p["conv_w"][perm], NBLK_F, np.float32)
    conv_b = _to_pblocks(p["conv_b"][perm][:, None], NBLK_F, np.float32)
    xproj_w = _to_pblocks(p["xproj_w"][perm], NBLK_F, BF16NP)
    dt_w = np.ascontiguousarray(p["dt_w"][:, sl_own].astype(BF16NP))
    dt_b = _to_pblocks(p["dt_b"][sl_own][:, None], NBLK_H, np.float32)
    A = _to_pblocks(-np.exp(p["A_log"][sl_own]), NBLK_H, np.float32)
    D = _to_pblocks(p["D_skip"][sl_own][:, None], NBLK_H, np.float32)

    fuse_half = fuse_w[:D_MODEL] if dr == 0 else fuse_w[D_MODEL:]
    w_out_full = p["out_w"].astype(np.float64) @ fuse_half.astype(np.float64)
    w_out = _to_pblocks(w_out_full[sl_own].astype(np.float32), 4, BF16NP)

    ident = np.eye(128, dtype=BF16NP)

    return {
        "xT": xT, "w_in": w_in, "w_z": w_z, "conv_w": conv_w,
        "conv_b": conv_b, "xproj_w": xproj_w, "dt_w": dt_w, "dt_b": dt_b,
        "A": A, "D": D, "w_out": w_out, "ident": ident,
    }


def kernel(_spmd_kwargs=None, **inputs):
    hs = np.asarray(inputs["hidden_states"], dtype=np.float32)
    fuse_w = np.asarray(inputs["fuse_w"], dtype=np.float32)
    fuse_b = np.asarray(inputs["fuse_b"], dtype=np.float32)
    params = []
    for pre in ("fwd_", "bwd_"):
        params.append({k[len(pre):]: np.asarray(v, dtype=np.float32)
                       for k, v in inputs.items() if k.startswith(pre)})

    nc = _get_program()

    in_maps = []
    core_cfg = []
    prep_cache = {}
    for c in range(8):
        b, dr, h = c >> 2, (c >> 1) & 1, c & 1
        core_cfg.append((b, dr, h))
        key = (b, dr, h)
        if key not in prep_cache:
            prep_cache[key] = _core_inputs(hs, params, fuse_w, b, dr, h)
        in_maps.append(prep_cache[key])

    res = run_bass_kernel_spmd(nc, in_maps, core_ids=list(range(8)),
                               **(_spmd_kwargs or {}))

    out = np.zeros((B, L, D_MODEL), dtype=np.float32)
    for c in range(8):
        b, dr, h = core_cfg[c]
        contrib = res.results[c]["out_part"].T  # (L, D_MODEL)
        if dr == 1:
            contrib = contrib[::-1]
        out[b] += contrib
    out += fuse_b[None, None, :]
    if _spmd_kwargs is not None:
        kernel._last_result = res
    return out


# revision 23
# speedup vs baseline: 1.2484x; 1.0027x over previous
"""BiMambaBlock on 8 Trainium2 NeuronCores.

Sharding: core c = (batch b, direction d, d_inner-half h) with
b = c>>2, d = (c>>1)&1, h = c&1.  Every core runs the same program on
different data (weights sliced/permuted per core on the host):

  - host feeds x[b].T in bf16 (flipped along L for bwd cores, padded
    with 4 leading zero cols for the causal conv), so the device always
    runs a *forward* mamba mixer in channels-on-partitions layout [d, L].
  - each core computes the full xc = silu(conv(x @ in_w_xi)) over all
    1024 channels (so the xproj contraction over d_inner stays local,
    no collectives), but scans only its 512-channel half (the host
    permutes weights so the own half is always channel blocks 0-3).
  - out_proj and the final fuse matmul are folded on the host into one
    [512ch, 512dm] weight; each core emits a partial [512dm, L] f32
    which the host transposes/flips/sums.

v3 structure.  The 64 DVE tensor_tensor_scans (4.42us each, 283us)
are the hard floor; everything else is arranged to overlap under them:
  - B_n/C_n rows round-trip through a DRAM scratch and are broadcast
    across partitions by stride-0-source DMAs (measured exact): no PE
    selector matmuls, no ACT drains, so every blk runs at DVE pace.
    C tiles (16) stay resident; B tiles rotate through 4 buffers with
    3-ahead prefetch.
  - phase 2 loops blk-outer / n-inner; y accumulates in PSUM via
    identity-lhsT matmuls on the otherwise idle PE (replaces 60 DVE
    adds); scan writes in-place over dA and the C-mul in-place over the
    scan output, with 2 dA buffers giving the PE reader slack.
  - the x padding is 4 columns so xi[:, 0:4] is exactly zero: the
    in_proj tail chunk (3 cols = 32 matmuls) becomes one memset, and
    the even conv-tap offsets stay 4B-aligned for DVE 4x tensor_scalar.
  - the z gate runs inside each blk's n-loop (xT persists; z weights
    are a separate small input), keeping all 64 z matmuls off the
    serial phase-1 prefix.
  - GPSIMD stays idle on purpose: measured tensor_tensor there is 5.9x
    slower AND degrades concurrent DVE scans ~50% via the shared port.
"""
import os
import sys

for _p in ("/opt/trn_rl_repo",):
    if os.path.isdir(_p) and _p not in sys.path:
        sys.path.insert(0, _p)

from contextlib import ExitStack

import ml_dtypes
import numpy as np

from concourse import bass, mybir, tile
from concourse.bass_utils import run_bass_kernel_spmd

F32 = mybir.dt.float32
BF16 = mybir.dt.bfloat16
AF = mybir.ActivationFunctionType
OP = mybir.AluOpType

D_MODEL = 512
D_INNER = 1024
DH = 512
N_STATE = 16
D_CONV = 4
DT_RANK = 32
B = 2
L = 2048
PAD = 4
LP = L + PAD

NBLK_F = D_INNER // 128  # 8 channel blocks for conv/xproj
NBLK_H = DH // 128       # 4 scan blocks

NCH = L // 512           # 512-wide matmul N-chunks

BF16NP = ml_dtypes.bfloat16


def _build_program():
    nc = bass.Bass(trn_type="TRN2", target_bir_lowering=False, debug=False)

    xT_d = nc.dram_tensor("xT", [128, 4 * LP], BF16, kind="ExternalInput")
    w_in_d = nc.dram_tensor("w_in", [128, 4 * 1024], BF16, kind="ExternalInput")
    w_z_d = nc.dram_tensor("w_z", [128, 4 * 512], BF16, kind="ExternalInput")
    conv_w_d = nc.dram_tensor("conv_w", [128, NBLK_F * D_CONV], F32, kind="ExternalInput")
    conv_b_d = nc.dram_tensor("conv_b", [128, NBLK_F], F32, kind="ExternalInput")
    xproj_w_d = nc.dram_tensor("xproj_w", [128, NBLK_F * 64], BF16, kind="ExternalInput")
    dt_w_d = nc.dram_tensor("dt_w", [DT_RANK, DH], BF16, kind="ExternalInput")
    dt_b_d = nc.dram_tensor("dt_b", [128, NBLK_H], F32, kind="ExternalInput")
    A_d = nc.dram_tensor("A", [128, NBLK_H * N_STATE], F32, kind="ExternalInput")
    D_d = nc.dram_tensor("D", [128, NBLK_H], F32, kind="ExternalInput")
    w_out_d = nc.dram_tensor("w_out", [128, 4 * D_MODEL], BF16, kind="ExternalInput")
    ident_d = nc.dram_tensor("ident", [128, 128], BF16, kind="ExternalInput")
    bcrows_d = nc.dram_tensor("bc_rows", [32, L], BF16, kind="ExternalOutput")
    out_d = nc.dram_tensor("out_part", [D_MODEL, L], F32, kind="ExternalOutput")

    with tile.TileContext(nc) as tc, ExitStack() as ctx:
        # ---------------- persistent tiles ----------------
        wp = ctx.enter_context(tc.tile_pool(name="weights", bufs=1))
        xT = wp.tile([128, 4 * LP], BF16, tag="xT")
        w_z = wp.tile([128, 4 * 512], BF16, tag="w_z")
        conv_w = wp.tile([128, NBLK_F * D_CONV], F32, tag="conv_w")
        conv_b = wp.tile([128, NBLK_F], F32, tag="conv_b")
        xproj_w = wp.tile([128, NBLK_F * 64], BF16, tag="xproj_w")
        dt_w = wp.tile([DT_RANK, DH], BF16, tag="dt_w")
        dt_b = wp.tile([128, NBLK_H], F32, tag="dt_b")
        A_sb = wp.tile([128, NBLK_H * N_STATE], F32, tag="A")
        D_sb = wp.tile([128, NBLK_H], F32, tag="D")
        w_out = wp.tile([128, 4 * D_MODEL], BF16, tag="w_out")
        ident = wp.tile([128, 128], BF16, tag="ident")

        glob = ctx.enter_context(tc.tile_pool(name="glob", bufs=1))
        xc_t = [glob.tile([128, L], BF16, tag=f"xc{i}", name=f"xc{i}")
                for i in range(NBLK_H)]  # own-half xc; reused as gate output
        dt_t = [glob.tile([128, L], BF16, tag=f"dt{i}", name=f"dt{i}")
                for i in range(NBLK_H)]
        gz = glob.tile([128, L], BF16, tag="gz")
        dbc = glob.tile([64, L], BF16, tag="dbc")

        pio = ctx.enter_context(tc.tile_pool(name="pio", bufs=2, space="PSUM"))
        ph2 = ctx.enter_context(tc.tile_pool(name="ph2", bufs=1))
        pdtx = ctx.enter_context(tc.tile_pool(name="pdtx", bufs=2))
        pda = ctx.enter_context(tc.tile_pool(name="pda", bufs=2))

        xT_v = xT[:].rearrange("p (k l) -> p k l", k=4)
        w_z_v = w_z[:].rearrange("p (k m) -> p k m", k=4)
        xproj_v = xproj_w[:].rearrange("p (k f) -> p k f", k=NBLK_F)

        def dt_chain(blk):
            """dt_t[blk] = softplus(dbc_dt @ dt_w + dt_b)."""
            dte = ph2.tile([128, L], BF16, tag="dte", name="dte")
            for nch in range(NCH):
                ps = pio.tile([128, 512], F32, tag="pio", name="ps_dt")
                nc.tensor.matmul(
                    ps[:], lhsT=dt_w[:, blk * 128:(blk + 1) * 128],
                    rhs=dbc[0:DT_RANK, nch * 512:(nch + 1) * 512],
                    start=True, stop=True)
                nc.scalar.activation(dte[:, nch * 512:(nch + 1) * 512],
                                     ps[:], AF.Exp, bias=dt_b[:, blk:blk + 1])
            nc.scalar.activation(dt_t[blk][:], dte[:], AF.Ln, bias=1.0)

        # ---------------- phase 1: xc / xproj ----------------
        with tc.tile_pool(name="pin", bufs=1) as pin, \
             tc.tile_pool(name="ph1b", bufs=2) as ph1b, \
             tc.tile_pool(name="ph1c", bufs=2) as ph1c, \
             tc.tile_pool(name="pio1", bufs=2, space="PSUM") as pio1, \
             tc.tile_pool(name="pdbc", bufs=1, space="PSUM") as pdbc:
            w_in = pin.tile([128, 4 * 1024], BF16, tag="w_in")

            # fine-grained DMA order: first matmul's operands land first
            for kb in range(4):
                nc.sync.dma_start(xT[:, kb * LP:kb * LP + PAD + 512],
                                  xT_d[:, kb * LP:kb * LP + PAD + 512])
            for kb in range(4):
                nc.sync.dma_start(w_in[:, kb * 1024:kb * 1024 + 512],
                                  w_in_d[:, kb * 1024:kb * 1024 + 512])
            for nch in range(1, 4):
                for kb in range(4):
                    o = kb * LP + PAD + nch * 512
                    nc.sync.dma_start(xT[:, o:o + 512], xT_d[:, o:o + 512])
            for kb in range(4):
                nc.sync.dma_start(w_in[:, kb * 1024 + 512:(kb + 1) * 1024],
                                  w_in_d[:, kb * 1024 + 512:(kb + 1) * 1024])
            for t, d in [(w_z, w_z_d), (conv_w, conv_w_d),
                         (conv_b, conv_b_d), (xproj_w, xproj_w_d),
                         (dt_w, dt_w_d), (dt_b, dt_b_d), (A_sb, A_d),
                         (D_sb, D_d), (w_out, w_out_d), (ident, ident_d)]:
                nc.sync.dma_start(t[:], d[:])

            w_in_v = w_in[:].rearrange("p (k m) -> p k m", k=4)
            ps_dbc = [pdbc.tile([64, 512], F32, tag=f"pdbc{nch}",
                                name=f"ps_dbc{nch}") for nch in range(NCH)]

            for blk in range(NBLK_F):
                xi = ph1b.tile([128, LP], BF16, tag="xi", name="xi")
                nc.vector.memset(xi[:, 0:PAD], 0.0)
                m0 = blk * 128
                for nch in range(NCH):
                    ps = pio1.tile([128, 512], F32, tag="pio1", name="ps_in")
                    for kb in range(4):
                        nc.tensor.matmul(
                            ps[:],
                            lhsT=w_in_v[:, kb, m0:m0 + 128],
                            rhs=xT_v[:, kb, PAD + nch * 512:PAD + (nch + 1) * 512],
                            start=(kb == 0), stop=(kb == 3),
                        )
                    nc.scalar.copy(xi[:, PAD + nch * 512:PAD + (nch + 1) * 512],
                                   ps[:])
                # conv: tap offsets 2/4 on DVE (4B-aligned -> 4x
                # tensor_scalar), offsets 1/3 on ACT, pair-adds on DVE.
                # offset k+1 carries conv weight k.
                acc = ph1c.tile([128, L], BF16, tag="ct0", name="ct0")
                nc.vector.tensor_scalar_mul(
                    acc[:], xi[:, 2:2 + L], conv_w[:, blk * 4 + 1:blk * 4 + 2])
                t2 = ph1c.tile([128, L], BF16, tag="ct2", name="ct2")
                nc.vector.tensor_scalar_mul(
                    t2[:], xi[:, 4:4 + L], conv_w[:, blk * 4 + 3:blk * 4 + 4])
                t1 = ph1c.tile([128, L], BF16, tag="ct1", name="ct1")
                nc.scalar.mul(t1[:], xi[:, 1:1 + L],
                              conv_w[:, blk * 4:blk * 4 + 1])
                t3 = ph1c.tile([128, L], BF16, tag="ct3", name="ct3")
                nc.scalar.mul(t3[:], xi[:, 3:3 + L],
                              conv_w[:, blk * 4 + 2:blk * 4 + 3])
                nc.vector.tensor_tensor(acc[:], acc[:], t2[:], OP.add)
                nc.vector.tensor_tensor(t1[:], t1[:], t3[:], OP.add)
                nc.vector.tensor_tensor(acc[:], acc[:], t1[:], OP.add)
                if blk < NBLK_H:
                    xc = xc_t[blk]
                else:
                    xc = ph1b.tile([128, L], BF16, tag="xcO", name=f"xcO{blk}")
                nc.scalar.activation(xc[:], acc[:], AF.Silu,
                                     bias=conv_b[:, blk:blk + 1])
                # xproj contribution (accumulates over all 8 blocks)
                for nch in range(NCH):
                    nc.tensor.matmul(
                        ps_dbc[nch][:], lhsT=xproj_v[:, blk, :],
                        rhs=xc[:, nch * 512:(nch + 1) * 512],
                        start=(blk == 0), stop=(blk == NBLK_F - 1),
                    )
                    if blk == NBLK_F - 1:
                        nc.scalar.copy(dbc[:, nch * 512:(nch + 1) * 512],
                                       ps_dbc[nch][:])

        # ------- transition: B/C rows -> DRAM, first broadcasts, dt0 -------
        # B rows first so fetch_B(0) can launch before the C rows land
        nc.sync.dma_start(bcrows_d[0:16, :], dbc[32:48, :])
        nc.sync.dma_start(bcrows_d[16:32, :], dbc[48:64, :])

        pct = ctx.enter_context(tc.tile_pool(name="pct", bufs=1))
        Ct_t = [pct.tile([128, L], BF16, tag=f"Ct{n}", name=f"Ct{n}")
                for n in range(N_STATE)]
        pbt = ctx.enter_context(tc.tile_pool(name="pbt", bufs=4))
        bt_slot = {}

        def fetch_B(g):
            """stride-0 DMA broadcast of B row (g = blk*16 + n)."""
            t = pbt.tile([128, L], BF16, tag="Bt", name=f"Bt{g}")
            nc.sync.dma_start(
                t[:], bcrows_d[g % 16:g % 16 + 1, :].to_broadcast([128, L]))
            bt_slot[g] = t

        def fetch_C(n):
            nc.sync.dma_start(
                Ct_t[n][:],
                bcrows_d[16 + n:16 + n + 1, :].to_broadcast([128, L]))

        for g in range(3):
            fetch_B(g)
            fetch_C(g)
        dt_chain(0)

        psy = ctx.enter_context(tc.tile_pool(name="psy", bufs=1, space="PSUM"))

        # -------- phase 2: blk-outer scan loop, y accumulated in PSUM -----
        for blk in range(NBLK_H):
            dtx = pdtx.tile([128, L], BF16, tag="dtx", name="dtx")
            nc.vector.tensor_tensor(dtx[:], dt_t[blk][:], xc_t[blk][:],
                                    OP.mult)
            ypsum = psy.tile([128, L], F32, tag="ypsum", name="ypsum")
            for n in range(N_STATE):
                g = blk * N_STATE + n
                dA = pda.tile([128, L], BF16, tag="dA", name="dA")
                nc.scalar.activation(
                    dA[:], dt_t[blk][:], AF.Exp,
                    scale=A_sb[:, blk * N_STATE + n:blk * N_STATE + n + 1])
                if g + 3 < NBLK_H * N_STATE:
                    fetch_B(g + 3)
                if blk == 0 and n + 3 < N_STATE:
                    fetch_C(n + 3)
                dBx = ph2.tile([128, L], BF16, tag="dBx", name="dBx")
                nc.vector.tensor_tensor(dBx[:], dtx[:], bt_slot.pop(g)[:],
                                        OP.mult)
                # scan overwrites dA; the C-mul overwrites the scan output
                nc.vector.tensor_tensor_scan(
                    dA[:], dA[:], dBx[:], 0.0, OP.mult, OP.add)
                nc.vector.tensor_tensor(dA[:], dA[:], Ct_t[n][:], OP.mult)
                for nch in range(NCH):
                    nc.tensor.matmul(
                        ypsum[:, nch * 512:(nch + 1) * 512],
                        lhsT=ident[:], rhs=dA[:, nch * 512:(nch + 1) * 512],
                        start=(n == 0), stop=False)
                # z-gate chunks and the next blk's dt chain ride inside
                # the n-loop: PE and ACT have slack under the scans.  All
                # four z chunks go at one n so ACT pays a single Silu
                # activation-table swap per blk instead of four.
                if n == 2:
                    for zn in range(NCH):
                        ps = pio.tile([128, 512], F32, tag="pio", name="ps_z")
                        for kb in range(4):
                            nc.tensor.matmul(
                                ps[:],
                                lhsT=w_z_v[:, kb, blk * 128:(blk + 1) * 128],
                                rhs=xT_v[:, kb, PAD + zn * 512:PAD + (zn + 1) * 512],
                                start=(kb == 0), stop=(kb == 3),
                            )
                        nc.scalar.activation(gz[:, zn * 512:(zn + 1) * 512],
                                             ps[:], AF.Silu)
                if n == 10 and blk + 1 < NBLK_H:
                    dt_chain(blk + 1)
            # D skip: y += D*xc via one more PE accumulation
            dxc = pda.tile([128, L], BF16, tag="dA", name="dxc")
            nc.scalar.mul(dxc[:], xc_t[blk][:], D_sb[:, blk:blk + 1])
            for nch in range(NCH):
                nc.tensor.matmul(
                    ypsum[:, nch * 512:(nch + 1) * 512],
                    lhsT=ident[:], rhs=dxc[:, nch * 512:(nch + 1) * 512],
                    start=False, stop=True)
            # gate: t = y * silu(z); overwrites xc (dead after dxc)
            nc.vector.tensor_tensor(xc_t[blk][:], ypsum[:], gz[:], OP.mult)

        # ---------------- phase 3: out-proj tail ----------------
        w_out_v = w_out[:].rearrange("p (k m) -> p k m", k=4)
        with tc.tile_pool(name="ph3b", bufs=2) as ph3b:
            for m in range(4):
                for nch in range(NCH):
                    ps = pio.tile([128, 512], F32, tag="pio", name="ps_out")
                    for kb in range(NBLK_H):
                        nc.tensor.matmul(
                            ps[:], lhsT=w_out_v[:, kb, m * 128:(m + 1) * 128],
                            rhs=xc_t[kb][:, nch * 512:(nch + 1) * 512],
                            start=(kb == 0), stop=(kb == NBLK_H - 1))
                    ob = ph3b.tile([128, 512], F32, tag="outb", name="outb")
                    nc.scalar.copy(ob[:], ps[:])
                    nc.sync.dma_start(
                        out_d[m * 128:(m + 1) * 128,
                              nch * 512:(nch + 1) * 512], ob[:])

    _split_excess_waits(nc)
    return nc


def _split_excess_waits(nc, max_waits=1):
    """The walrus build rejects instructions carrying more than one
    sync-wait command ("Too many sync wait commands" on Tile's kernel-tail
    Drain, which waits on every loose semaphore). Move excess waits onto
    NoOps placed just before the offender on the same engine."""
    for fn in nc.m.functions:
        for blk in fn.blocks:
            out, changed = [], False
            for inst in blk.instructions:
                si = inst.sync_info
                waits = list(si.on_wait) if si is not None and si.on_wait else []
                if len(waits) > max_waits:
                    extra, keep = waits[:-max_waits], waits[-max_waits:]
                    chunks = [extra[i:i + max_waits]
                              for i in range(0, len(extra), max_waits)]
                    for j, ch in enumerate(chunks):
                        nop = mybir.InstNoOp(
                            name=f"{inst.name}-waitsplit{j}", ins=[], outs=[])
                        nop.engine = inst.engine
                        nop.sync_info = mybir.SyncInfo(on_wait=ch, on_update=[])
                        out.append(nop)
                    si.on_wait = keep
                    changed = True
                out.append(inst)
            if changed:
                blk.instructions = out
    return nc


_PROG = None


def _get_program():
    global _PROG
    if _PROG is None:
        _PROG = _build_program()
    return _PROG


def _to_pblocks(a, nblk, dtype):
    """[nblk*128, f] -> [128, nblk*f] with [p, blk*f+j] = a[blk*128+p, j]."""
    a = np.ascontiguousarray(a)
    f = a.shape[1] if a.ndim > 1 else 1
    a = a.reshape(nblk, 128, f).transpose(1, 0, 2).reshape(128, nblk * f)
    return np.ascontiguousarray(a.astype(dtype))


def _core_inputs(hs, params, fuse_w, b, dr, h):
    p = params[dr]
    x = hs[b]
    if dr == 1:
        x = x[::-1]
    xTp = np.concatenate(
        [np.zeros((D_MODEL, PAD), np.float32), np.ascontiguousarray(x.T)],
        axis=1)
    xT = _to_pblocks(xTp, 4, BF16NP)  # [128, 4*(L+PAD)] bf16

    sl_own = slice(h * DH, (h + 1) * DH)
    perm = np.r_[h * DH:(h + 1) * DH, (1 - h) * DH:(2 - h) * DH]

    in_w = p["in_w"]
    w_in = _to_pblocks(in_w[:, :D_INNER][:, perm], 4, BF16NP)
    w_z = _to_pblocks(in_w[:, D_INNER:][:, sl_own], 4, BF16NP)

    conv_w = _to_pblocks(p["conv_w"][perm], NBLK_F, np.float32)
    conv_b = _to_pblocks(p["conv_b"][perm][:, None], NBLK_F, np.float32)
    xproj_w = _to_pblocks(p["xproj_w"][perm], NBLK_F, BF16NP)
    dt_w = np.ascontiguousarray(p["dt_w"][:, sl_own].astype(BF16NP))
    dt_b = _to_pblocks(p["dt_b"][sl_own][:, None], NBLK_H, np.float32)
    A = _to_pblocks(-np.exp(p["A_log"][sl_own]), NBLK_H, np.float32)
    D = _to_pblocks(p["D_skip"][sl_own][:, None], NBLK_H, np.float32)

    fuse_half = fuse_w[:D_MODEL] if dr == 0 else fuse_w[D_MODEL:]
    w_out_full = p["out_w"].astype(np.float64) @ fuse_half.astype(np.float64)
    w_out = _to_pblocks(w_out_full[sl_own].astype(np.float32), 4, BF16NP)

    ident = np.eye(128, dtype=BF16NP)

    return {
        "xT": xT, "w_in": w_in, "w_z": w_z, "conv_w": conv_w,
        "conv_b": conv_b, "xproj_w": xproj_w, "dt_w": dt_w, "dt_b": dt_b,
        "A": A, "D": D, "w_out": w_out, "ident": ident,
    }


def kernel(_spmd_kwargs=None, **inputs):
    hs = np.asarray(inputs["hidden_states"], dtype=np.float32)
    fuse_w = np.asarray(inputs["fuse_w"], dtype=np.float32)
    fuse_b = np.asarray(inputs["fuse_b"], dtype=np.float32)
    params = []
    for pre in ("fwd_", "bwd_"):
        params.append({k[len(pre):]: np.asarray(v, dtype=np.float32)
                       for k, v in inputs.items() if k.startswith(pre)})

    nc = _get_program()

    in_maps = []
    core_cfg = []
    prep_cache = {}
    for c in range(8):
        b, dr, h = c >> 2, (c >> 1) & 1, c & 1
        core_cfg.append((b, dr, h))
        key = (b, dr, h)
        if key not in prep_cache:
            prep_cache[key] = _core_inputs(hs, params, fuse_w, b, dr, h)
        in_maps.append(prep_cache[key])

    res = run_bass_kernel_spmd(nc, in_maps, core_ids=list(range(8)),
                               **(_spmd_kwargs or {}))

    out = np.zeros((B, L, D_MODEL), dtype=np.float32)
    for c in range(8):
        b, dr, h = core_cfg[c]
        contrib = res.results[c]["out_part"].T  # (L, D_MODEL)
        if dr == 1:
            contrib = contrib[::-1]
        out[b] += contrib
    out += fuse_b[None, None, :]
    if _spmd_kwargs is not None:
        kernel._last_result = res
    return out
